# revision 49
# baseline (speedup 1.0000x reference)
"""Joint soft-histogram kernel for Trainium2 (Bass/Tile), 8-core data parallel.

Math (per batch b, K=256, L=1/256, W=L/2.5, N=65536 pixels):
    phi_k(x) = S_k(x) - S_{k+1}(x),   S_k(x) = sigmoid(640*x - 2.5*k)
    out[k, j] = sum_n phi_k(x_n) * phi_j(y_n) / N

Double telescope: out = Drow(Dcol(M)) / N with M = Sx^T @ Sy (257 x 257),
M[k, j] = sum_n S_k(x_n) * S_j(y_n). Neither side needs a per-chunk adjacent
difference -- both collapse onto the tiny M. M entries grow to O(N), so PSUM
fp32 accumulation is drained to SBUF every SEG chunks (caps entries at
SEG*128 = 8192, keeping roundoff ~1e-3 absolute, ~4e-3 relative after
differencing -- inside the 2e-2 budget).

Engine plan (v4, measured ~334us vs 461us baseline; engine busy: PE ~284,
ACT ~266, DVE ~263, GPSIMD ~99 -- three-engine balanced):
  - preadd A[p, c*KP+j] = 640*v[p,c] - 2.5*j: ONE broadcast-AP tensor_tensor
    per 16-chunk group (4.4us/group, 1x mode -- stride-0 APs forbid 2x, and
    fp16 operands would lose too much argument precision), writing SBUF.
    v1 did this as 1024 per-chunk TENSOR_SCALARs = 484us.
  - sigmoid: one big staged ACTIVATE per group (3.7us/group).
  - y-side preadd groups are spread across engines to balance load: 8 on
    GPSIMD TT (14.4us/group there), 8 as per-chunk fused ACTIVATE-with-bias
    on ScalarE (~400ns/chunk, no preadd at all), 16 on DVE. More than ~8 on
    GPSIMD makes it the critical path.
  - PE: per chunk 2x 128-row matmuls + 1-row tail matmul (row 256), fp16.
    Accumulation is parity-split across two independent PSUM chains so
    consecutive matmuls never RAW-depend on the same bank. PE preadds (v2:
    rank-2 matmuls [ones;v]^T[kr;ones]) measured 2.4x slower than modeled
    and forced small PSUM-sourced ACTIVATEs -- abandoned.
  - epilogue row-diff via PE with a bidiagonal matrix (DVE cannot read
    partition-shifted operands).

Sharding: pure data parallel, batch b -> core b.
"""

import numpy as np

import concourse.bass as bass
import concourse.tile as tile
from concourse import bacc, mybir
from concourse.bass_utils import run_bass_kernel_spmd

F32 = mybir.dt.float32
F16 = mybir.dt.float16

B = 8
K = 256
KB = K + 1            # 257 sigmoid taps per side (k = 0..256)
KP = K + 2            # 258: per-chunk stride in staged tiles (even)
NPIX = 65536
NCHUNK = 512
XG = 16               # chunks per staged group
NG = NCHUNK // XG     # 32 groups
GF = XG * KP          # staged group free size (4128)
INV_N = 1.0 / NPIX
SEG = 64              # chunks per PSUM accumulation segment
NSEG = NCHUNK // SEG

# --- tuning knobs -----------------------------------------------------------
# Preadd engine per (group, side): 'v' = DVE broadcast-TT, 'g' = GPSIMD TT,
# 'a' = per-chunk fused ACTIVATE with per-partition bias (no preadd at all).
X_ENG = ['v'] * NG
Y_ENG = [('g' if g % 4 == 2 else ('a' if g % 8 == 0 else 'v'))
         for g in range(NG)]
# ---------------------------------------------------------------------------

_cached_nc = None


def _build():
    nc = bacc.Bacc("TRN2")
    xd = nc.declare_dram_parameter("x", [128, 512], F32, isOutput=False)
    yd = nc.declare_dram_parameter("y", [128, 512], F32, isOutput=False)
    kd = nc.declare_dram_parameter("krow", [128, KP], F32, isOutput=False)
    # dmat[k, k'] = [k==k'] - [k==k'+1]; dnext[k, k'] = -[k==0][k'==127]
    dmd = nc.declare_dram_parameter("dmat", [128, 128], F32, isOutput=False)
    dnd = nc.declare_dram_parameter("dnext", [128, 128], F32, isOutput=False)
    od = nc.declare_dram_parameter("out", [256, 256], F32, isOutput=True)

    sig = mybir.ActivationFunctionType.Sigmoid
    add = mybir.AluOpType.add

    with tile.TileContext(nc) as tc:
        with (
            tc.tile_pool(name="singles", bufs=1) as singles,
            tc.tile_pool(name="stage32", bufs=3) as stage32,
            tc.tile_pool(name="stage16", bufs=4) as stage16,
            tc.tile_pool(name="work", bufs=5) as work,
            tc.tile_pool(name="psum", bufs=1, space="PSUM") as psum,
        ):
            xt = singles.tile([128, 512], F32)
            nc.sync.dma_start(out=xt, in_=xd[:, :])
            yt = singles.tile([128, 512], F32)
            nc.sync.dma_start(out=yt, in_=yd[:, :])
            kr = singles.tile([128, KP], F32)
            nc.sync.dma_start(out=kr, in_=kd[:, :])
            dm = singles.tile([128, 128], F32)
            nc.sync.dma_start(out=dm, in_=dmd[:, :])
            dn = singles.tile([128, 128], F32)
            nc.sync.dma_start(out=dn, in_=dnd[:, :])

            # M accumulators in SBUF: rows 0..127 / 128..255 / 256 (tail)
            acc = singles.tile([128, 2, KB], F32)
            acct = singles.tile([128, KB], F32)  # only partition 0 used
            nc.vector.memset(acc, 0.0)
            nc.vector.memset(acct[0:1, :], 0.0)

            # PSUM: M' segment accumulators (rows 0..255 + tail row 256),
            # split by chunk parity into two independent accumulation chains
            # so consecutive matmuls never RAW-depend on the same bank.
            Mp0 = psum.tile([128, 2, 512], F32, tag="mp0")
            Mp1 = psum.tile([128, 2, 512], F32, tag="mp1")
            Mt0 = psum.tile([128, 512], F32, tag="mt0")
            Mt1 = psum.tile([128, 512], F32, tag="mt1")
            Mp = [Mp0, Mp1]
            Mt = [Mt0, Mt1]

            def preadd_sigmoid(src, g, eng, tag, pieces=1):
                # pieces>1 splits the preadd+sigmoid into smaller units so
                # the first matmuls can start sooner (startup ramp).
                a = stage32.tile([128, XG, KP], F32, tag="a" + tag)
                s = stage16.tile([128, XG, KP], F16, tag="s" + tag)
                tt = nc.gpsimd.tensor_tensor if eng == 'g' else \
                    nc.vector.tensor_tensor
                w = XG // pieces
                for p in range(pieces):
                    lo, hi = p * w, (p + 1) * w
                    tt(
                        out=a[:, lo:hi, :],
                        in0=src[:, g * XG + lo:g * XG + hi].unsqueeze(2)
                            .broadcast_to([128, w, KP]),
                        in1=kr.unsqueeze(1).broadcast_to([128, w, KP]),
                        op=add,
                    )
                    nc.scalar.activation(
                        out=s[:, lo:hi, :], in_=a[:, lo:hi, :], func=sig,
                    )
                return s

            for g in range(NG):
                npc = 4 if g <= 1 else 1
                sx = preadd_sigmoid(xt, g, X_ENG[g], "x", pieces=npc)
                fused_y = Y_ENG[g] == 'a'
                if not fused_y:
                    sy = preadd_sigmoid(yt, g, Y_ENG[g], "y", pieces=npc)
                for i in range(XG):
                    c = g * XG + i
                    sb = c % 2
                    first = c % SEG == sb
                    last = c % SEG == SEG - 2 + sb
                    if fused_y:
                        tyt = work.tile([128, KB], F16, tag="tyf")
                        nc.scalar.activation(
                            out=tyt, in_=kr[:, 0:KB], func=sig,
                            bias=yt[:, c:c + 1], scale=1.0,
                        )
                        ty = tyt[:, :]
                    else:
                        ty = sy[:, i, 0:KB]
                    nc.tensor.matmul(
                        Mp[sb][:, 0, 0:KB],
                        lhsT=sx[:, i, 0:128],
                        rhs=ty,
                        start=first,
                        stop=last,
                    )
                    nc.tensor.matmul(
                        Mp[sb][:, 1, 0:KB],
                        lhsT=sx[:, i, 128:256],
                        rhs=ty,
                        start=first,
                        stop=last,
                    )
                    nc.tensor.matmul(
                        Mt[sb][0:1, 0:KB],
                        lhsT=sx[:, i, 256:257],
                        rhs=ty,
                        start=first,
                        stop=last,
                    )
                    if last:
                        # drain this parity's segment into SBUF accumulators
                        for h in range(2):
                            nc.vector.tensor_add(
                                out=acc[:, h, :], in0=acc[:, h, :],
                                in1=Mp[sb][:, h, 0:KB],
                            )
                        nc.vector.tensor_add(
                            out=acct[0:1, :], in0=acct[0:1, :],
                            in1=Mt[sb][0:1, 0:KB],
                        )

            # Epilogue: out[k, j] = (Mr[k, j] - Mr[k, j+1]) / N with
            # Mr[k, j] = acc[k, j] - acc[k+1, j], row diff via PE:
            # rd_h = dmat^T @ acc_h + dnext^T @ acc_{h+1}.
            for h in range(2):
                rd = psum.tile([128, 512], F32, tag="rd")
                nc.tensor.matmul(
                    rd[:, 0:KB], lhsT=dm, rhs=acc[:, h, :],
                    start=True, stop=False,
                )
                nxt = acc[:, 1, :] if h == 0 else acct[:, :]
                nc.tensor.matmul(
                    rd[:, 0:KB], lhsT=dn, rhs=nxt,
                    start=False, stop=True,
                )
                t1 = work.tile([128, KB], F32, tag="ep")
                nc.scalar.activation(
                    out=t1, in_=rd[:, 0:KB],
                    func=mybir.ActivationFunctionType.Copy, scale=INV_N,
                )
                t2 = work.tile([128, K], F32, tag="ep2")
                nc.vector.tensor_sub(out=t2, in0=t1[:, 0:K], in1=t1[:, 1:KB])
                nc.sync.dma_start(out=od[128 * h: 128 * (h + 1), :], in_=t2)

    nc.finalize()
    return nc


def _get_nc():
    global _cached_nc
    if _cached_nc is None:
        _cached_nc = _build()
    return _cached_nc


def _krow():
    row = np.arange(KP, dtype=np.float32) * np.float32(-2.5)
    return np.tile(row[None, :], (128, 1))


def _dmat():
    d = np.eye(128, dtype=np.float32)
    d -= np.eye(128, k=-1, dtype=np.float32)
    return d


def _dnext():
    d = np.zeros((128, 128), dtype=np.float32)
    d[0, 127] = -1.0
    return d


def _in_maps(x, y):
    x = np.asarray(x, dtype=np.float32)
    y = np.asarray(y, dtype=np.float32)
    kr = _krow()
    maps = []
    for b in range(B):
        x6 = np.ascontiguousarray(x[b].reshape(128, 512) * np.float32(640.0))
        y6 = np.ascontiguousarray(y[b].reshape(128, 512) * np.float32(640.0))
        maps.append({"x": x6, "y": y6, "krow": kr,
                     "dmat": _dmat(), "dnext": _dnext()})
    return maps


def run(x, y, trace=False, **trace_kw):
    """Run on all 8 cores; returns (out (8,256,256) f32, BassKernelResults)."""
    nc = _get_nc()
    res = run_bass_kernel_spmd(nc, _in_maps(x, y), list(range(B)), trace=trace,
                               **trace_kw)
    out = np.stack([res.results[b]["out"] for b in range(B)]).astype(np.float32)
    return out, res


def kernel(x, y):
    out, _ = run(x, y)
    return out


# revision 51
# speedup vs baseline: 1.0021x; 1.0021x over previous
"""Joint soft-histogram kernel for Trainium2 (Bass/Tile), 8-core data parallel.

Math (per batch b, K=256, L=1/256, W=L/2.5, N=65536 pixels):
    phi_k(x) = S_k(x) - S_{k+1}(x),   S_k(x) = sigmoid(640*x - 2.5*k)
    out[k, j] = sum_n phi_k(x_n) * phi_j(y_n) / N

Double telescope: out = Drow(Dcol(M)) / N with M = Sx^T @ Sy (257 x 257),
M[k, j] = sum_n S_k(x_n) * S_j(y_n). Neither side needs a per-chunk adjacent
difference -- both collapse onto the tiny M. M entries grow to O(N), so PSUM
fp32 accumulation is drained to SBUF every SEG chunks (caps entries at
SEG*128 = 8192, keeping roundoff ~1e-3 absolute, ~4e-3 relative after
differencing -- inside the 2e-2 budget).

Engine plan (v5, measured ~328us vs 461us baseline; engine busy: PE ~283,
ACT ~265, DVE ~262, GPSIMD ~95 -- three-engine balanced, ~45us of
pipeline-ramp/drain gaps):
  - preadd A[p, c*KP+j] = 640*v[p,c] - 2.5*j: ONE broadcast-AP tensor_tensor
    per 16-chunk group (4.4us/group, 1x mode -- stride-0 APs forbid 2x, and
    fp16 operands would lose too much argument precision), writing SBUF.
    v1 did this as 1024 per-chunk TENSOR_SCALARs = 484us.
  - sigmoid: one big staged ACTIVATE per group (3.7us/group).
  - y-side preadd groups are spread across engines to balance load: 8 on
    GPSIMD TT (14.4us/group there), 4 as per-chunk fused ACTIVATE-with-bias
    on ScalarE (~400ns/chunk, no preadd at all), 20 on DVE. More GPSIMD
    groups or more fused groups make those engines the critical path.
  - groups 0-1 stage their preadd+sigmoid in 4-chunk pieces so the first
    matmuls start ~5us earlier (startup ramp).
  - PE: per chunk 2x 128-row matmuls + 1-row tail matmul (row 256), fp16.
    Accumulation is parity-split across two independent PSUM chains so
    consecutive matmuls never RAW-depend on the same bank. PE preadds (v2:
    rank-2 matmuls [ones;v]^T[kr;ones]) measured 2.4x slower than modeled
    and forced small PSUM-sourced ACTIVATEs -- abandoned.
  - epilogue row-diff via PE with a bidiagonal matrix (DVE cannot read
    partition-shifted operands).

Sharding: pure data parallel, batch b -> core b.
"""

import numpy as np

import concourse.bass as bass
import concourse.tile as tile
from concourse import bacc, mybir
from concourse.bass_utils import run_bass_kernel_spmd

F32 = mybir.dt.float32
F16 = mybir.dt.float16

B = 8
K = 256
KB = K + 1            # 257 sigmoid taps per side (k = 0..256)
KP = K + 2            # 258: per-chunk stride in staged tiles (even)
NPIX = 65536
NCHUNK = 512
XG = 16               # chunks per staged group
NG = NCHUNK // XG     # 32 groups
GF = XG * KP          # staged group free size (4128)
INV_N = 1.0 / NPIX
SEG = 64              # chunks per PSUM accumulation segment
NSEG = NCHUNK // SEG

# --- tuning knobs -----------------------------------------------------------
# Preadd engine per (group, side): 'v' = DVE broadcast-TT, 'g' = GPSIMD TT,
# 'a' = per-chunk fused ACTIVATE with per-partition bias (no preadd at all).
X_ENG = ['v'] * NG
Y_ENG = [('g' if g % 4 == 2 else ('a' if g % 8 == 0 else 'v'))
         for g in range(NG)]
# ---------------------------------------------------------------------------

_cached_nc = None


def _build():
    nc = bacc.Bacc("TRN2")
    xd = nc.declare_dram_parameter("x", [128, 512], F32, isOutput=False)
    yd = nc.declare_dram_parameter("y", [128, 512], F32, isOutput=False)
    kd = nc.declare_dram_parameter("krow", [128, KP], F32, isOutput=False)
    # dmat[k, k'] = [k==k'] - [k==k'+1]; dnext[k, k'] = -[k==0][k'==127]
    dmd = nc.declare_dram_parameter("dmat", [128, 128], F32, isOutput=False)
    dnd = nc.declare_dram_parameter("dnext", [128, 128], F32, isOutput=False)
    od = nc.declare_dram_parameter("out", [256, 256], F32, isOutput=True)

    sig = mybir.ActivationFunctionType.Sigmoid
    add = mybir.AluOpType.add

    with tile.TileContext(nc) as tc:
        with (
            tc.tile_pool(name="singles", bufs=1) as singles,
            tc.tile_pool(name="stage32", bufs=3) as stage32,
            tc.tile_pool(name="stage16", bufs=4) as stage16,
            tc.tile_pool(name="work", bufs=5) as work,
            tc.tile_pool(name="psum", bufs=1, space="PSUM") as psum,
        ):
            # Preload the sigmoid ACT table-set (~2.7us) while DMAs run:
            # memset a tiny tile, then a 1-wide dummy sigmoid.
            warm = singles.tile([128, 2], F32)
            nc.vector.memset(warm, 0.0)
            nc.scalar.activation(out=warm, in_=warm, func=sig)

            xt = singles.tile([128, 512], F32)
            nc.sync.dma_start(out=xt, in_=xd[:, :])
            yt = singles.tile([128, 512], F32)
            nc.sync.dma_start(out=yt, in_=yd[:, :])
            kr = singles.tile([128, KP], F32)
            nc.sync.dma_start(out=kr, in_=kd[:, :])
            dm = singles.tile([128, 128], F32)
            nc.sync.dma_start(out=dm, in_=dmd[:, :])
            dn = singles.tile([128, 128], F32)
            nc.sync.dma_start(out=dn, in_=dnd[:, :])

            # M accumulators in SBUF: rows 0..127 / 128..255 / 256 (tail)
            acc = singles.tile([128, 2, KB], F32)
            acct = singles.tile([128, KB], F32)  # only partition 0 used
            nc.vector.memset(acc, 0.0)
            nc.vector.memset(acct[0:1, :], 0.0)

            # PSUM: M' segment accumulators (rows 0..255 + tail row 256),
            # split by chunk parity into two independent accumulation chains
            # so consecutive matmuls never RAW-depend on the same bank.
            Mp0 = psum.tile([128, 2, 512], F32, tag="mp0")
            Mp1 = psum.tile([128, 2, 512], F32, tag="mp1")
            Mt0 = psum.tile([128, 512], F32, tag="mt0")
            Mt1 = psum.tile([128, 512], F32, tag="mt1")
            Mp = [Mp0, Mp1]
            Mt = [Mt0, Mt1]

            def preadd_sigmoid(src, g, eng, tag, pieces=1):
                # pieces>1 splits the preadd+sigmoid into smaller units so
                # the first matmuls can start sooner (startup ramp).
                a = stage32.tile([128, XG, KP], F32, tag="a" + tag)
                s = stage16.tile([128, XG, KP], F16, tag="s" + tag)
                tt = nc.gpsimd.tensor_tensor if eng == 'g' else \
                    nc.vector.tensor_tensor
                w = XG // pieces
                for p in range(pieces):
                    lo, hi = p * w, (p + 1) * w
                    tt(
                        out=a[:, lo:hi, :],
                        in0=src[:, g * XG + lo:g * XG + hi].unsqueeze(2)
                            .broadcast_to([128, w, KP]),
                        in1=kr.unsqueeze(1).broadcast_to([128, w, KP]),
                        op=add,
                    )
                    nc.scalar.activation(
                        out=s[:, lo:hi, :], in_=a[:, lo:hi, :], func=sig,
                    )
                return s

            for g in range(NG):
                npc = 4 if g <= 1 else 1
                sx = preadd_sigmoid(xt, g, X_ENG[g], "x", pieces=npc)
                fused_y = Y_ENG[g] == 'a'
                if not fused_y:
                    sy = preadd_sigmoid(yt, g, Y_ENG[g], "y", pieces=npc)
                for i in range(XG):
                    c = g * XG + i
                    sb = c % 2
                    first = c % SEG == sb
                    last = c % SEG == SEG - 2 + sb
                    if fused_y:
                        tyt = work.tile([128, KB], F16, tag="tyf")
                        nc.scalar.activation(
                            out=tyt, in_=kr[:, 0:KB], func=sig,
                            bias=yt[:, c:c + 1], scale=1.0,
                        )
                        ty = tyt[:, :]
                    else:
                        ty = sy[:, i, 0:KB]
                    nc.tensor.matmul(
                        Mp[sb][:, 0, 0:KB],
                        lhsT=sx[:, i, 0:128],
                        rhs=ty,
                        start=first,
                        stop=last,
                    )
                    nc.tensor.matmul(
                        Mp[sb][:, 1, 0:KB],
                        lhsT=sx[:, i, 128:256],
                        rhs=ty,
                        start=first,
                        stop=last,
                    )
                    nc.tensor.matmul(
                        Mt[sb][0:1, 0:KB],
                        lhsT=sx[:, i, 256:257],
                        rhs=ty,
                        start=first,
                        stop=last,
                    )
                    if last:
                        # drain this parity's segment into SBUF accumulators
                        for h in range(2):
                            nc.vector.tensor_add(
                                out=acc[:, h, :], in0=acc[:, h, :],
                                in1=Mp[sb][:, h, 0:KB],
                            )
                        nc.vector.tensor_add(
                            out=acct[0:1, :], in0=acct[0:1, :],
                            in1=Mt[sb][0:1, 0:KB],
                        )

            # Epilogue: out[k, j] = (Mr[k, j] - Mr[k, j+1]) / N with
            # Mr[k, j] = acc[k, j] - acc[k+1, j], row diff via PE:
            # rd_h = dmat^T @ acc_h + dnext^T @ acc_{h+1}.
            for h in range(2):
                rd = psum.tile([128, 512], F32, tag="rd")
                nc.tensor.matmul(
                    rd[:, 0:KB], lhsT=dm, rhs=acc[:, h, :],
                    start=True, stop=False,
                )
                nxt = acc[:, 1, :] if h == 0 else acct[:, :]
                nc.tensor.matmul(
                    rd[:, 0:KB], lhsT=dn, rhs=nxt,
                    start=False, stop=True,
                )
                t1 = work.tile([128, KB], F32, tag="ep")
                nc.vector.tensor_copy(out=t1, in_=rd[:, 0:KB])
                t2 = work.tile([128, K], F32, tag="ep2")
                nc.vector.tensor_sub(out=t2, in0=t1[:, 0:K], in1=t1[:, 1:KB])
                nc.sync.dma_start(out=od[128 * h: 128 * (h + 1), :], in_=t2)

    nc.finalize()
    return nc


def _get_nc():
    global _cached_nc
    if _cached_nc is None:
        _cached_nc = _build()
    return _cached_nc


def _krow():
    row = np.arange(KP, dtype=np.float32) * np.float32(-2.5)
    return np.tile(row[None, :], (128, 1))


def _dmat():
    d = np.eye(128, dtype=np.float32)
    d -= np.eye(128, k=-1, dtype=np.float32)
    return d * np.float32(INV_N)


def _dnext():
    d = np.zeros((128, 128), dtype=np.float32)
    d[0, 127] = -1.0
    return d * np.float32(INV_N)


def _in_maps(x, y):
    x = np.asarray(x, dtype=np.float32)
    y = np.asarray(y, dtype=np.float32)
    kr = _krow()
    maps = []
    for b in range(B):
        x6 = np.ascontiguousarray(x[b].reshape(128, 512) * np.float32(640.0))
        y6 = np.ascontiguousarray(y[b].reshape(128, 512) * np.float32(640.0))
        maps.append({"x": x6, "y": y6, "krow": kr,
                     "dmat": _dmat(), "dnext": _dnext()})
    return maps


def run(x, y, trace=False, **trace_kw):
    """Run on all 8 cores; returns (out (8,256,256) f32, BassKernelResults)."""
    nc = _get_nc()
    res = run_bass_kernel_spmd(nc, _in_maps(x, y), list(range(B)), trace=trace,
                               **trace_kw)
    out = np.stack([res.results[b]["out"] for b in range(B)]).astype(np.float32)
    return out, res


def kernel(x, y):
    out, _ = run(x, y)
    return out


# revision 52
# speedup vs baseline: 1.0241x; 1.0220x over previous
"""Joint soft-histogram kernel for Trainium2 (Bass/Tile), 8-core data parallel.

Math (per batch b, K=256, L=1/256, W=L/2.5, N=65536 pixels):
    phi_k(x) = S_k(x) - S_{k+1}(x),   S_k(x) = sigmoid(640*x - 2.5*k)
    out[k, j] = sum_n phi_k(x_n) * phi_j(y_n) / N

Double telescope: out = Drow(Dcol(M)) / N with M = Sx^T @ Sy (257 x 257),
M[k, j] = sum_n S_k(x_n) * S_j(y_n). Neither side needs a per-chunk adjacent
difference -- both collapse onto the tiny M. M entries grow to O(N), so PSUM
fp32 accumulation is drained to SBUF every SEG chunks (caps entries at
SEG*128 = 8192, keeping roundoff ~1e-3 absolute, ~4e-3 relative after
differencing -- inside the 2e-2 budget).

Engine plan (v5, measured ~328us vs 461us baseline; engine busy: PE ~283,
ACT ~265, DVE ~262, GPSIMD ~95 -- three-engine balanced, ~45us of
pipeline-ramp/drain gaps):
  - preadd A[p, c*KP+j] = 640*v[p,c] - 2.5*j: ONE broadcast-AP tensor_tensor
    per 16-chunk group (4.4us/group, 1x mode -- stride-0 APs forbid 2x, and
    fp16 operands would lose too much argument precision), writing SBUF.
    v1 did this as 1024 per-chunk TENSOR_SCALARs = 484us.
  - sigmoid: one big staged ACTIVATE per group (3.7us/group).
  - y-side preadd groups are spread across engines to balance load: 8 on
    GPSIMD TT (14.4us/group there), 4 as per-chunk fused ACTIVATE-with-bias
    on ScalarE (~400ns/chunk, no preadd at all), 20 on DVE. More GPSIMD
    groups or more fused groups make those engines the critical path.
  - groups 0-1 stage their preadd+sigmoid in 4-chunk pieces so the first
    matmuls start ~5us earlier (startup ramp).
  - PE: per chunk 2x 128-row matmuls + 1-row tail matmul (row 256), fp16.
    Accumulation is parity-split across two independent PSUM chains so
    consecutive matmuls never RAW-depend on the same bank. PE preadds (v2:
    rank-2 matmuls [ones;v]^T[kr;ones]) measured 2.4x slower than modeled
    and forced small PSUM-sourced ACTIVATEs -- abandoned.
  - epilogue row-diff via PE with a bidiagonal matrix (DVE cannot read
    partition-shifted operands).

Sharding: pure data parallel, batch b -> core b.
"""

import numpy as np

import concourse.bass as bass
import concourse.tile as tile
from concourse import bacc, mybir
from concourse.bass_utils import run_bass_kernel_spmd

F32 = mybir.dt.float32
F16 = mybir.dt.float16

B = 8
K = 256
KB = K + 1            # 257 sigmoid taps per side (k = 0..256)
KP = K + 2            # 258: per-chunk stride in staged tiles (even)
NPIX = 65536
NCHUNK = 512
XG = 16               # chunks per staged group
NG = NCHUNK // XG     # 32 groups
GF = XG * KP          # staged group free size (4128)
INV_N = 1.0 / NPIX
SEG = 64              # chunks per PSUM accumulation segment
NSEG = NCHUNK // SEG

# --- tuning knobs -----------------------------------------------------------
# Preadd engine per (group, side): 'v' = DVE broadcast-TT, 'g' = GPSIMD TT,
# 'a' = per-chunk fused ACTIVATE with per-partition bias (no preadd at all).
X_ENG = ['v'] * NG
Y_ENG = [('g' if g % 4 == 2 else ('a' if g % 8 == 0 else 'v'))
         for g in range(NG)]
# ---------------------------------------------------------------------------

_cached_nc = None


def _build():
    nc = bacc.Bacc("TRN2")
    xd = nc.declare_dram_parameter("x", [128, 512], F32, isOutput=False)
    yd = nc.declare_dram_parameter("y", [128, 512], F32, isOutput=False)
    kd = nc.declare_dram_parameter("krow", [128, KP], F32, isOutput=False)
    # dmat[k, k'] = [k==k'] - [k==k'+1]; dnext[k, k'] = -[k==0][k'==127]
    dmd = nc.declare_dram_parameter("dmat", [128, 128], F32, isOutput=False)
    dnd = nc.declare_dram_parameter("dnext", [128, 128], F32, isOutput=False)
    od = nc.declare_dram_parameter("out", [256, 256], F32, isOutput=True)

    sig = mybir.ActivationFunctionType.Sigmoid
    add = mybir.AluOpType.add

    with tile.TileContext(nc) as tc:
        with (
            tc.tile_pool(name="singles", bufs=1) as singles,
            tc.tile_pool(name="stage32", bufs=3) as stage32,
            tc.tile_pool(name="stage16", bufs=4) as stage16,
            tc.tile_pool(name="work", bufs=5) as work,
            tc.tile_pool(name="psum", bufs=1, space="PSUM") as psum,
        ):
            # Preload the sigmoid ACT table-set (~2.7us) while DMAs run:
            # memset a tiny tile, then a 1-wide dummy sigmoid.
            warm = singles.tile([128, 2], F32)
            nc.vector.memset(warm, 0.0)
            nc.scalar.activation(out=warm, in_=warm, func=sig)

            xt = singles.tile([128, 512], F32)
            nc.sync.dma_start(out=xt, in_=xd[:, :])
            yt = singles.tile([128, 512], F32)
            nc.sync.dma_start(out=yt, in_=yd[:, :])
            kr = singles.tile([128, KP], F32)
            nc.sync.dma_start(out=kr, in_=kd[:, :])
            dm = singles.tile([128, 128], F32)
            nc.sync.dma_start(out=dm, in_=dmd[:, :])
            dn = singles.tile([128, 128], F32)
            nc.sync.dma_start(out=dn, in_=dnd[:, :])

            # M accumulators in SBUF: rows 0..127 / 128..255 / 256 (tail)
            acc = singles.tile([128, 2, KB], F32)
            acct = singles.tile([128, KB], F32)  # only partition 0 used
            nc.vector.memset(acc, 0.0)
            nc.vector.memset(acct[0:1, :], 0.0)

            # PSUM: M' segment accumulators (rows 0..255 + tail row 256).
            # Segment s uses chain s%3 (s%2 for the tail), so each chain's
            # drain has 2 segments (~80us) of slack before its banks restart
            # -- the old parity-split left only ~1 chunk of slack and stalled
            # the PE at every segment boundary.
            Mp0 = psum.tile([128, 2, 512], F32, tag="mp0")
            Mp1 = psum.tile([128, 2, 512], F32, tag="mp1")
            Mp2 = psum.tile([128, 2, 512], F32, tag="mp2")
            Mt0 = psum.tile([128, 512], F32, tag="mt0")
            Mt1 = psum.tile([128, 512], F32, tag="mt1")
            Mp = [Mp0, Mp1, Mp2]
            Mt = [Mt0, Mt1]

            def preadd_sigmoid(src, g, eng, tag, pieces=1):
                # pieces>1 splits the preadd+sigmoid into smaller units so
                # the first matmuls can start sooner (startup ramp).
                a = stage32.tile([128, XG, KP], F32, tag="a" + tag)
                s = stage16.tile([128, XG, KP], F16, tag="s" + tag)
                tt = nc.gpsimd.tensor_tensor if eng == 'g' else \
                    nc.vector.tensor_tensor
                w = XG // pieces
                for p in range(pieces):
                    lo, hi = p * w, (p + 1) * w
                    tt(
                        out=a[:, lo:hi, :],
                        in0=src[:, g * XG + lo:g * XG + hi].unsqueeze(2)
                            .broadcast_to([128, w, KP]),
                        in1=kr.unsqueeze(1).broadcast_to([128, w, KP]),
                        op=add,
                    )
                    nc.scalar.activation(
                        out=s[:, lo:hi, :], in_=a[:, lo:hi, :], func=sig,
                    )
                return s

            for g in range(NG):
                npc = 4 if g <= 1 else 1
                sx = preadd_sigmoid(xt, g, X_ENG[g], "x", pieces=npc)
                fused_y = Y_ENG[g] == 'a'
                if not fused_y:
                    sy = preadd_sigmoid(yt, g, Y_ENG[g], "y", pieces=npc)
                for i in range(XG):
                    c = g * XG + i
                    seg = c // SEG
                    sb = seg % 3
                    tb = seg % 2
                    first = c % SEG == 0
                    last = c % SEG == SEG - 1
                    if fused_y:
                        tyt = work.tile([128, KB], F16, tag="tyf")
                        nc.scalar.activation(
                            out=tyt, in_=kr[:, 0:KB], func=sig,
                            bias=yt[:, c:c + 1], scale=1.0,
                        )
                        ty = tyt[:, :]
                    else:
                        ty = sy[:, i, 0:KB]
                    nc.tensor.matmul(
                        Mp[sb][:, 0, 0:KB],
                        lhsT=sx[:, i, 0:128],
                        rhs=ty,
                        start=first,
                        stop=last,
                    )
                    nc.tensor.matmul(
                        Mp[sb][:, 1, 0:KB],
                        lhsT=sx[:, i, 128:256],
                        rhs=ty,
                        start=first,
                        stop=last,
                    )
                    nc.tensor.matmul(
                        Mt[tb][0:1, 0:KB],
                        lhsT=sx[:, i, 256:257],
                        rhs=ty,
                        start=first,
                        stop=last,
                    )
                    if last:
                        # drain this segment into SBUF accumulators
                        for h in range(2):
                            nc.vector.tensor_add(
                                out=acc[:, h, :], in0=acc[:, h, :],
                                in1=Mp[sb][:, h, 0:KB],
                            )
                        nc.vector.tensor_add(
                            out=acct[0:1, :], in0=acct[0:1, :],
                            in1=Mt[tb][0:1, 0:KB],
                        )

            # Epilogue: out[k, j] = (Mr[k, j] - Mr[k, j+1]) / N with
            # Mr[k, j] = acc[k, j] - acc[k+1, j], row diff via PE:
            # rd_h = dmat^T @ acc_h + dnext^T @ acc_{h+1}.
            for h in range(2):
                # reuse a dead accumulator bank for the epilogue (PSUM full)
                rd = Mp[h][:, 0, :]
                nc.tensor.matmul(
                    rd[:, 0:KB], lhsT=dm, rhs=acc[:, h, :],
                    start=True, stop=False,
                )
                nxt = acc[:, 1, :] if h == 0 else acct[:, :]
                nc.tensor.matmul(
                    rd[:, 0:KB], lhsT=dn, rhs=nxt,
                    start=False, stop=True,
                )
                t1 = work.tile([128, KB], F32, tag="ep")
                nc.vector.tensor_copy(out=t1, in_=rd[:, 0:KB])
                t2 = work.tile([128, K], F32, tag="ep2")
                nc.vector.tensor_sub(out=t2, in0=t1[:, 0:K], in1=t1[:, 1:KB])
                nc.sync.dma_start(out=od[128 * h: 128 * (h + 1), :], in_=t2)

    nc.finalize()
    return nc


def _get_nc():
    global _cached_nc
    if _cached_nc is None:
        _cached_nc = _build()
    return _cached_nc


def _krow():
    row = np.arange(KP, dtype=np.float32) * np.float32(-2.5)
    return np.tile(row[None, :], (128, 1))


def _dmat():
    d = np.eye(128, dtype=np.float32)
    d -= np.eye(128, k=-1, dtype=np.float32)
    return d * np.float32(INV_N)


def _dnext():
    d = np.zeros((128, 128), dtype=np.float32)
    d[0, 127] = -1.0
    return d * np.float32(INV_N)


def _in_maps(x, y):
    x = np.asarray(x, dtype=np.float32)
    y = np.asarray(y, dtype=np.float32)
    kr = _krow()
    maps = []
    for b in range(B):
        x6 = np.ascontiguousarray(x[b].reshape(128, 512) * np.float32(640.0))
        y6 = np.ascontiguousarray(y[b].reshape(128, 512) * np.float32(640.0))
        maps.append({"x": x6, "y": y6, "krow": kr,
                     "dmat": _dmat(), "dnext": _dnext()})
    return maps


def run(x, y, trace=False, **trace_kw):
    """Run on all 8 cores; returns (out (8,256,256) f32, BassKernelResults)."""
    nc = _get_nc()
    res = run_bass_kernel_spmd(nc, _in_maps(x, y), list(range(B)), trace=trace,
                               **trace_kw)
    out = np.stack([res.results[b]["out"] for b in range(B)]).astype(np.float32)
    return out, res


def kernel(x, y):
    out, _ = run(x, y)
    return out
